# revision 8
# baseline (speedup 1.0000x reference)
"""Trainium2 Bass kernel for nn_MultiHeadAttention (B=2, S=2048, D=1024, H=16).

Sharding: 8 cores = 2 batches x 4 head-groups. Core c handles batch c//4 and
heads [4*(c%4), 4*(c%4)+4). Each core computes its 4 heads' attention plus the
row-slice of the output projection; the host sums the 4 partial outputs per
batch and adds the output bias.

Dataflow (cost model: matmul = N_out cycles regardless of M/K, so every
matmul keeps M=128 / K=128 where the math allows):
  - qT/kT in [head_dim, seq] layout, 2 heads per 128-partition tile.
  - scoresT[kv, q] = kT.T @ qT per (head, kv-pair, q-chunk); exp on ScalarE
    (scale=1/8 folded in) -> ex tiles [128 kv, 1024] bf16. The exp stream is
    the ACT-side bottleneck and paces the steady-state blocks.
  - attn[q, d+1] = ex.T @ [v | 1]: ex is the stationary operand (M=128 q,
    K=128 kv, N=65), accumulated over 16 kv tiles into PSUM [128, 4*65]
    per (head, q-chunk); col 64 of each head block = softmax denominator.
  - DVE reciprocal + per-partition tensor_scalar_mul normalizes into
    attn_n [128 q, 4*64] bf16 (q on partitions = denominators are
    per-partition scalars, no broadcast matmuls needed).
  - PE transpose (identity matmul) flips head-pairs [128 q, 128] ->
    [128 (2h*d), 128 q]; out = at.T @ wo accumulates K=128 (2 heads) per
    pass, halving the output projection.
  - Emission is a software pipeline of blocks (qc, h): burst(qc, h) [attn@V
    for the previous stage] runs first, then se(qc+1, h, ·) [scores+exp]
    woven with projections/tails so PE and ACT both stay dense. The last
    stage's bursts ride inside the last score blocks so only the output
    projections trail the exp stream.

All matmuls run in bf16 (inputs cast on host) with fp32 PSUM accumulation.
"""

import sys

for _p in ("/opt/trn_rl_repo",):
    if _p not in sys.path:
        sys.path.insert(0, _p)

import numpy as np
import ml_dtypes

BF16 = ml_dtypes.bfloat16

S = 2048          # sequence length
D = 1024          # embed dim
HC = 4            # heads per core
HD = 64           # head dim
DC = HC * HD      # per-core projection width (256)
DT = D // 128     # D-tiles (8)
QC = S // 512     # q-chunks of 512 (4)
NKV = S // 128    # kv tiles of 128 (16)
NCORES = 8

_PROGRAM = None


def _build_program():
    import concourse.mybir as mybir
    import concourse.tile as tile
    from concourse import bacc

    dt = mybir.dt
    AF = mybir.ActivationFunctionType
    ALU = mybir.AluOpType

    nc = bacc.Bacc()

    xqT = nc.declare_dram_parameter("xqT", [D, S], dt.bfloat16, isOutput=False)
    xkT = nc.declare_dram_parameter("xkT", [D, S], dt.bfloat16, isOutput=False)
    xvT = nc.declare_dram_parameter("xvT", [D, S], dt.bfloat16, isOutput=False)
    wq = nc.declare_dram_parameter("wq", [D, DC], dt.bfloat16, isOutput=False)
    wk = nc.declare_dram_parameter("wk", [D, DC], dt.bfloat16, isOutput=False)
    wv = nc.declare_dram_parameter("wv", [D, DC], dt.bfloat16, isOutput=False)
    wo = nc.declare_dram_parameter("wo", [128, 2, D], dt.bfloat16, isOutput=False)
    bq = nc.declare_dram_parameter("bq", [128, 2], dt.float32, isOutput=False)
    bk = nc.declare_dram_parameter("bk", [128, 2], dt.float32, isOutput=False)
    bv = nc.declare_dram_parameter("bv", [128, DC], dt.float32, isOutput=False)
    ident = nc.declare_dram_parameter("ident", [128, 128], dt.bfloat16, isOutput=False)
    out = nc.declare_dram_parameter("out", [S, D], dt.float32, isOutput=True)

    out_t = out.rearrange("(t p) d -> t p d", p=128)

    with tile.TileContext(nc) as tc:
        with (
            tc.tile_pool(name="const", bufs=1) as cp,
            tc.tile_pool(name="xt", bufs=12) as xp,
            tc.tile_pool(name="xv", bufs=8) as xvp,
            tc.tile_pool(name="expp", bufs=32) as ep,
            tc.tile_pool(name="anp", bufs=8) as np_,
            tc.tile_pool(name="atp", bufs=4) as ap_,
            tc.tile_pool(name="rcp", bufs=2) as rp,
            tc.tile_pool(name="outp", bufs=2) as op_,
            tc.tile_pool(name="pa", bufs=2, space="PSUM") as pa,
            tc.tile_pool(name="pacc", bufs=2, space="PSUM") as pacc,
            tc.tile_pool(name="pmix", bufs=2, space="PSUM") as pm,
        ):
            # ---- persistent tiles ----
            wq_sb = cp.tile([128, DT, DC], dt.bfloat16, tag="wq_sb")
            wk_sb = cp.tile([128, DT, DC], dt.bfloat16, tag="wk_sb")
            wv_sb = cp.tile([128, DT, DC], dt.bfloat16, tag="wv_sb")
            wo_sb = cp.tile([128, 2, D], dt.bfloat16, tag="wo_sb")
            bq_sb = cp.tile([128, 2], dt.float32, tag="bq_sb")
            bk_sb = cp.tile([128, 2], dt.float32, tag="bk_sb")
            bv_sb = cp.tile([128, DC], dt.float32, tag="bv_sb")
            id_sb = cp.tile([128, 128], dt.bfloat16, tag="id_sb")

            qT_sb = [cp.tile([128, 2, 512], dt.bfloat16, tag=f"qT{i}", name=f"qT{i}")
                     for i in range(QC)]
            kT_sb = [cp.tile([128, 2, 512], dt.bfloat16, tag=f"kT{i}", name=f"kT{i}")
                     for i in range(QC)]
            # v' blocks of 65 per head: v cols 0..63, ones col 64
            v_sb = [cp.tile([128, HC * 65], dt.bfloat16, tag=f"v{i}", name=f"v{i}")
                    for i in range(NKV)]

            # x staged as eighth-tiles [128, DT, 256] so projection chains can
            # start as soon as the first bytes land
            xq_t: list = [None] * 8
            xk_t: list = [None] * 8
            xv_t: list = [None] * 8

            def dma_x(xT, arr, e, nm, pool=None):
                t = (pool or xp).tile([128, DT, 256], dt.bfloat16, tag="xt",
                                      name=f"x_{nm}{e}")
                nc.sync.dma_start(
                    t[:],
                    xT.rearrange("(t p) s -> p t s", p=128)[:, :, e * 256:(e + 1) * 256])
                arr[e] = t

            def dma_w(w_sb, w, pt):
                nc.sync.dma_start(
                    w_sb[:, :, pt * 128:(pt + 1) * 128],
                    w.rearrange("(t p) m -> p t m", p=128)[:, :, pt * 128:(pt + 1) * 128])

            def kq_proj(xts, w_sb, dst, b_sb, qc, pt):
                ps = pm.tile([128, 512], dt.float32, tag="pm", name=f"pp{qc}_{pt}")
                for pc in range(2):
                    for Dti in range(DT):
                        nc.tensor.matmul(
                            ps[:, pc * 256:(pc + 1) * 256],
                            w_sb[:, Dti, pt * 128:(pt + 1) * 128],
                            xts[qc * 2 + pc][:, Dti, :],
                            start=(Dti == 0),
                            stop=(Dti == DT - 1),
                        )
                nc.vector.tensor_scalar_add(dst[qc][:, pt, :], ps[:], b_sb[:, pt:pt + 1])

            def v_chain(st, h):
                e, off = st // 2, (st % 2) * 128
                ps = pm.tile([128, HD], dt.float32, tag="pm", name=f"vp{st}_{h}")
                for Dti in range(DT):
                    nc.tensor.matmul(
                        ps[:],
                        xv_t[e][:, Dti, off:off + 128],
                        wv_sb[:, Dti, h * HD:(h + 1) * HD],
                        start=(Dti == 0),
                        stop=(Dti == DT - 1),
                    )
                nc.vector.tensor_tensor(
                    v_sb[st].rearrange("p (h c) -> p h c", c=65)[:, h, 0:64],
                    ps[:], bv_sb[:, h * HD:(h + 1) * HD], ALU.add)

            exs = {}

            def se(qc, h, kvb):
                pt, lo = h // 2, (h % 2) * 64
                scp = pa.tile([128, 1024], dt.float32, tag="pa", name=f"sc{qc}_{h}_{kvb}")
                for j in range(2):
                    kt = kvb * 2 + j
                    nc.tensor.matmul(
                        scp[:, j * 512:(j + 1) * 512],
                        kT_sb[kt // 4][lo:lo + 64, pt, (kt % 4) * 128:(kt % 4 + 1) * 128],
                        qT_sb[qc][lo:lo + 64, pt, :],
                        start=True,
                        stop=True,
                    )
                ex = ep.tile([128, 1024], dt.bfloat16, tag="ex", name=f"ex{qc}_{h}_{kvb}")
                nc.scalar.activation(ex[:], scp[:], AF.Exp, scale=0.125)
                exs[(qc, h, kvb)] = ex

            attn_n = {}

            def burst(qc, h):
                # attn[q, d]+sums for (qc, h), all 4 q-subtiles, K accumulated
                # over the 16 kv tiles; ex is the stationary operand.
                acc = pacc.tile([128, HC * 65], dt.float32, tag="acc", name=f"acc{qc}_{h}")
                for j in range(4):
                    for kt in range(NKV):
                        e = exs[(qc, h, kt // 2)]
                        o = (kt % 2) * 512 + j * 128
                        nc.tensor.matmul(
                            acc[:, j * 65:j * 65 + 65],
                            e[:, o:o + 128],
                            v_sb[kt][:, h * 65:(h + 1) * 65],
                            start=(kt == 0),
                            stop=(kt == NKV - 1),
                        )
                for kvb in range(8):
                    exs.pop((qc, h, kvb))
                if h == 0:
                    for j in range(4):
                        attn_n[(qc, j)] = np_.tile(
                            [128, 256], dt.bfloat16, tag="an", name=f"an{qc}_{j}")
                accv = acc.rearrange("p (j c) -> p j c", c=65)
                rc = rp.tile([128, HC], dt.float32, tag="rc", name=f"rc{qc}_{h}")
                nc.vector.reciprocal(rc[:], accv[:, :, 64])
                for j in range(4):
                    nc.vector.tensor_scalar_mul(
                        attn_n[(qc, j)][:, h * 64:(h + 1) * 64],
                        accv[:, j, 0:64], rc[:, j:j + 1])

            at_t = {}

            def tr_j(qc, j):
                # transpose head pairs of q-subtile j: [128 q, 128] -> psum,
                # drained to at_t as the out-proj stationary
                att = attn_n[(qc, j)]
                att_t = ap_.tile([128, 256], dt.bfloat16, tag="at", name=f"at{qc}_{j}")
                for hp in range(2):
                    tr = pm.tile([128, 128], dt.bfloat16, tag="pm", name=f"tr{qc}_{j}_{hp}")
                    nc.tensor.transpose(tr[:], att[:, hp * 128:(hp + 1) * 128], id_sb[:])
                    nc.vector.tensor_copy(att_t[:, hp * 128:(hp + 1) * 128], tr[:])
                at_t[(qc, j)] = att_t

            def op_j(qc, j):
                # output projection row-slice for s-tile qc*4+j
                att_t = at_t.pop((qc, j))
                st = qc * 4 + j
                o_sb = op_.tile([128, D], dt.float32, tag="osb", name=f"o{st}")
                for dc2 in range(2):
                    po = pm.tile([128, 512], dt.float32, tag="pm", name=f"po{st}_{dc2}")
                    for hp in range(2):
                        nc.tensor.matmul(
                            po[:],
                            att_t[:, hp * 128:(hp + 1) * 128],
                            wo_sb[:, hp, dc2 * 512:(dc2 + 1) * 512],
                            start=(hp == 0),
                            stop=(hp == 1),
                        )
                    nc.vector.tensor_copy(o_sb[:, dc2 * 512:(dc2 + 1) * 512], po[:])
                    nc.sync.dma_start(
                        out_t[st][:, dc2 * 512:(dc2 + 1) * 512],
                        o_sb[:, dc2 * 512:(dc2 + 1) * 512])

            def tails(qc):
                # pipelined transpose -> out-proj works for a q-chunk; caller
                # weaves them between se units so the DVE drains are hidden
                return [
                    lambda qc=qc: tr_j(qc, 0),
                    lambda qc=qc: (op_j(qc, 0), tr_j(qc, 1)),
                    lambda qc=qc: (op_j(qc, 1), tr_j(qc, 2)),
                    lambda qc=qc: (op_j(qc, 2), tr_j(qc, 3)),
                    lambda qc=qc: op_j(qc, 3),
                ]

            def weave(ses, works):
                # emit se units with works spread evenly between them
                n, m = len(ses), len(works)
                if n == 0:
                    for w in works:
                        w()
                    return
                wi = 0
                for i, s in enumerate(ses):
                    se(*s)
                    while wi < m and (wi + 1) * n <= (i + 1) * m:
                        works[wi]()
                        wi += 1
                while wi < m:
                    works[wi]()
                    wi += 1

            # ---- DMA emission (SP queue, consumption order; k/q first) ----
            dma_w(wk_sb, wk, 0)
            dma_x(xkT, xk_t, 0, "k")
            dma_x(xkT, xk_t, 1, "k")
            nc.sync.dma_start(bk_sb[:], bk[:])
            dma_w(wq_sb, wq, 0)
            dma_x(xqT, xq_t, 0, "q")
            dma_x(xqT, xq_t, 1, "q")
            nc.sync.dma_start(bq_sb[:], bq[:])
            for e in range(2, 8):
                dma_x(xkT, xk_t, e, "k")
            dma_x(xqT, xq_t, 2, "q")
            dma_x(xqT, xq_t, 3, "q")
            dma_w(wk_sb, wk, 1)
            dma_w(wq_sb, wq, 1)
            dma_w(wv_sb, wv, 0)
            dma_w(wv_sb, wv, 1)
            nc.sync.dma_start(bv_sb[:], bv[:])
            nc.sync.dma_start(wo_sb[:], wo[:])
            nc.sync.dma_start(id_sb[:], ident[:])
            for e in range(8):
                dma_x(xvT, xv_t, e, "v", pool=xvp)
            for e in range(4, 8):
                dma_x(xqT, xq_t, e, "q")

            for st in range(NKV):
                nc.vector.memset(
                    v_sb[st].rearrange("p (h c) -> p h c", c=65)[:, :, 64:65], 1.0)

            # ---- window 0: stage(0) + projections (pt0 first: heads 0/1
            # only read the pt0 half of kT/qT) ----
            kq_proj(xk_t, wk_sb, kT_sb, bk_sb, 0, 0)
            kq_proj(xq_t, wq_sb, qT_sb, bq_sb, 0, 0)
            se(0, 0, 0); se(0, 0, 1)
            kq_proj(xk_t, wk_sb, kT_sb, bk_sb, 1, 0)
            se(0, 0, 2); se(0, 0, 3)
            kq_proj(xk_t, wk_sb, kT_sb, bk_sb, 2, 0)
            se(0, 0, 4); se(0, 0, 5)
            kq_proj(xk_t, wk_sb, kT_sb, bk_sb, 3, 0)
            se(0, 0, 6); se(0, 0, 7)
            kq_proj(xq_t, wq_sb, qT_sb, bq_sb, 1, 0)
            # h1 block: vchains for head 0
            weave([(0, 1, kvb) for kvb in range(8)],
                  [lambda st=st: v_chain(st, 0) for st in range(NKV)])
            kq_proj(xk_t, wk_sb, kT_sb, bk_sb, 0, 1)
            kq_proj(xk_t, wk_sb, kT_sb, bk_sb, 1, 1)
            kq_proj(xq_t, wq_sb, qT_sb, bq_sb, 0, 1)
            # h2 block: vchains head 1 + remaining pt1 kT chunks
            w_h2 = [lambda st=st: v_chain(st, 1) for st in range(6)]
            w_h2 += [lambda: kq_proj(xk_t, wk_sb, kT_sb, bk_sb, 2, 1)]
            w_h2 += [lambda st=st: v_chain(st, 1) for st in range(6, 10)]
            w_h2 += [lambda: kq_proj(xk_t, wk_sb, kT_sb, bk_sb, 3, 1)]
            w_h2 += [lambda st=st: v_chain(st, 1) for st in range(10, NKV)]
            weave([(0, 2, kvb) for kvb in range(8)], w_h2)
            # h3 block: vchains head 2 + qT1 pt1
            w_h3 = [lambda st=st: v_chain(st, 2) for st in range(NKV)]
            w_h3.insert(4, lambda: kq_proj(xq_t, wq_sb, qT_sb, bq_sb, 1, 1))
            weave([(0, 3, kvb) for kvb in range(8)], w_h3)

            # ---- steady blocks (qc, h): burst(qc, h) + se(qc+1, h, ·) ----
            for qc in range(3):
                for h in range(HC):
                    burst(qc, h)
                    works = []
                    if qc == 0 and h == 0:
                        works += [lambda st=st: v_chain(st, 3) for st in range(NKV)]
                    if h == 0 and qc >= 1:
                        works.append(
                            lambda q=qc + 1: kq_proj(xq_t, wq_sb, qT_sb, bq_sb, q, 1))
                    if h == 3:
                        works += tails(qc)
                        if qc < 2:
                            works.append(
                                lambda q=qc + 2: kq_proj(xq_t, wq_sb, qT_sb, bq_sb, q, 0))
                    weave([(qc + 1, h, kvb) for kvb in range(8)], works)
                    if qc == 2:
                        # immediately consume the stage we just emitted: the
                        # last q-chunk's attn@V rides the exp stream instead
                        # of trailing it
                        burst(3, h)
            for w in tails(3):
                w()

    nc.finalize()
    return nc


def _get_program():
    global _PROGRAM
    if _PROGRAM is None:
        _PROGRAM = _build_program()
    return _PROGRAM


def _prep_core_inputs(x_q, x_k, x_v, wq, bq, wk, bk, wv, bv, wo):
    """Build the 8 per-core input dicts (host-side shard + cast)."""
    ident_np = np.eye(128, dtype=np.float32).astype(BF16)
    xT = {}
    for b in range(2):
        xT[b] = (
            np.ascontiguousarray(x_q[b].T).astype(BF16),
            np.ascontiguousarray(x_k[b].T).astype(BF16),
            np.ascontiguousarray(x_v[b].T).astype(BF16),
        )
    in_maps = []
    for c in range(NCORES):
        b, g = c // 4, c % 4
        sl = slice(g * DC, (g + 1) * DC)
        # wo rows for this head group, stacked per head pair: row hh*64+d of
        # pair hp = wo row for head 2*hp+hh, dim d
        wo_c = np.ascontiguousarray(
            wo[sl, :].reshape(2, 2, HD, D).transpose(1, 2, 0, 3).reshape(128, 2, D)
        ).astype(BF16)
        in_maps.append({
            "xqT": xT[b][0],
            "xkT": xT[b][1],
            "xvT": xT[b][2],
            "wq": wq[:, sl].astype(BF16),
            "wk": wk[:, sl].astype(BF16),
            "wv": wv[:, sl].astype(BF16),
            "wo": wo_c,
            "bq": np.ascontiguousarray(bq[sl].reshape(2, 128).T).astype(np.float32),
            "bk": np.ascontiguousarray(bk[sl].reshape(2, 128).T).astype(np.float32),
            "bv": np.broadcast_to(bv[sl], (128, DC)).astype(np.float32).copy(),
            "ident": ident_np,
        })
    return in_maps


def kernel(x_q, x_k, x_v, wq, bq, wk, bk, wv, bv, wo, bo):
    from concourse.bass_utils import run_bass_kernel_spmd

    x_q = np.asarray(x_q, np.float32)
    x_k = np.asarray(x_k, np.float32)
    x_v = np.asarray(x_v, np.float32)
    wq = np.asarray(wq, np.float32)
    wk = np.asarray(wk, np.float32)
    wv = np.asarray(wv, np.float32)
    wo = np.asarray(wo, np.float32)
    bq = np.asarray(bq, np.float32)
    bk = np.asarray(bk, np.float32)
    bv = np.asarray(bv, np.float32)
    bo = np.asarray(bo, np.float32)

    nc = _get_program()
    in_maps = _prep_core_inputs(x_q, x_k, x_v, wq, bq, wk, bk, wv, bv, wo)
    res = run_bass_kernel_spmd(nc, in_maps, list(range(NCORES)))

    out = np.zeros((2, S, D), np.float32)
    for c in range(NCORES):
        out[c // 4] += res.results[c]["out"]
    out += bo
    return out


# revision 15
# speedup vs baseline: 1.0003x; 1.0003x over previous
"""Trainium2 Bass kernel for nn_MultiHeadAttention (B=2, S=2048, D=1024, H=16).

Sharding: 8 cores = 2 batches x 4 head-groups. Core c handles batch c//4 and
heads [4*(c%4), 4*(c%4)+4). Each core computes its 4 heads' attention plus the
row-slice of the output projection; the host sums the 4 partial outputs per
batch and adds the output bias.

Dataflow (cost model: matmul = N_out cycles regardless of M/K, so every
matmul keeps M=128 / K=128 where the math allows):
  - qT/kT in [head_dim, seq] layout, 2 heads per 128-partition tile.
  - scoresT[kv, q] = kT.T @ qT per (head, kv-pair, q-chunk); exp on ScalarE
    (scale=1/8 folded in) -> ex tiles [128 kv, 1024] bf16. The exp stream is
    the ACT-side bottleneck and paces the steady-state blocks.
  - attn[q, d+1] = ex.T @ [v | 1]: ex is the stationary operand (M=128 q,
    K=128 kv, N=65), accumulated over 16 kv tiles into PSUM [128, 4*65]
    per (head, q-chunk); col 64 of each head block = softmax denominator.
  - DVE reciprocal + per-partition tensor_scalar_mul normalizes into
    attn_n [128 q, 4*64] bf16 (q on partitions = denominators are
    per-partition scalars, no broadcast matmuls needed).
  - PE transpose (identity matmul) flips head-pairs [128 q, 128] ->
    [128 (2h*d), 128 q]; out = at.T @ wo accumulates K=128 (2 heads) per
    pass, halving the output projection.
  - Emission is a software pipeline of blocks (qc, h): burst(qc, h) [attn@V
    for the previous stage] runs first, then se(qc+1, h, ·) [scores+exp]
    woven with projections/tails so PE and ACT both stay dense. The last
    stage's bursts ride inside the last score blocks so only the output
    projections trail the exp stream.

All matmuls run in bf16 (inputs cast on host) with fp32 PSUM accumulation.
"""

import sys

for _p in ("/opt/trn_rl_repo",):
    if _p not in sys.path:
        sys.path.insert(0, _p)

import numpy as np
import ml_dtypes

BF16 = ml_dtypes.bfloat16

S = 2048          # sequence length
D = 1024          # embed dim
HC = 4            # heads per core
HD = 64           # head dim
DC = HC * HD      # per-core projection width (256)
DT = D // 128     # D-tiles (8)
QC = S // 512     # q-chunks of 512 (4)
NKV = S // 128    # kv tiles of 128 (16)
NCORES = 8

_PROGRAM = None


def _build_program():
    import concourse.mybir as mybir
    import concourse.tile as tile
    from concourse import bacc

    dt = mybir.dt
    AF = mybir.ActivationFunctionType
    ALU = mybir.AluOpType

    nc = bacc.Bacc()

    xqT = nc.declare_dram_parameter("xqT", [D, S], dt.bfloat16, isOutput=False)
    xkT = nc.declare_dram_parameter("xkT", [D, S], dt.bfloat16, isOutput=False)
    xvT = nc.declare_dram_parameter("xvT", [D, S], dt.bfloat16, isOutput=False)
    wq = nc.declare_dram_parameter("wq", [D, DC], dt.bfloat16, isOutput=False)
    wk = nc.declare_dram_parameter("wk", [D, DC], dt.bfloat16, isOutput=False)
    wv = nc.declare_dram_parameter("wv", [D, DC], dt.bfloat16, isOutput=False)
    wo = nc.declare_dram_parameter("wo", [128, 2, D], dt.bfloat16, isOutput=False)
    bq = nc.declare_dram_parameter("bq", [128, 2], dt.float32, isOutput=False)
    bk = nc.declare_dram_parameter("bk", [128, 2], dt.float32, isOutput=False)
    bv = nc.declare_dram_parameter("bv", [128, DC], dt.float32, isOutput=False)
    ident = nc.declare_dram_parameter("ident", [128, 128], dt.bfloat16, isOutput=False)
    out = nc.declare_dram_parameter("out", [S, D], dt.float32, isOutput=True)

    out_t = out.rearrange("(t p) d -> t p d", p=128)

    with tile.TileContext(nc) as tc:
        with (
            tc.tile_pool(name="const", bufs=1) as cp,
            tc.tile_pool(name="xt", bufs=12) as xp,
            tc.tile_pool(name="xv", bufs=8) as xvp,
            tc.tile_pool(name="expp", bufs=32) as ep,
            tc.tile_pool(name="anp", bufs=8) as np_,
            tc.tile_pool(name="atp", bufs=4) as ap_,
            tc.tile_pool(name="rcp", bufs=2) as rp,
            tc.tile_pool(name="outp", bufs=2) as op_,
            tc.tile_pool(name="pa", bufs=2, space="PSUM") as pa,
            tc.tile_pool(name="pacc", bufs=2, space="PSUM") as pacc,
            tc.tile_pool(name="pmix", bufs=2, space="PSUM") as pm,
        ):
            # ---- persistent tiles ----
            wq_sb = cp.tile([128, DT, DC], dt.bfloat16, tag="wq_sb")
            wk_sb = cp.tile([128, DT, DC], dt.bfloat16, tag="wk_sb")
            wv_sb = cp.tile([128, DT, DC], dt.bfloat16, tag="wv_sb")
            wo_sb = cp.tile([128, 2, D], dt.bfloat16, tag="wo_sb")
            bq_sb = cp.tile([128, 2], dt.float32, tag="bq_sb")
            bk_sb = cp.tile([128, 2], dt.float32, tag="bk_sb")
            bv_sb = cp.tile([128, DC], dt.float32, tag="bv_sb")
            id_sb = cp.tile([128, 128], dt.bfloat16, tag="id_sb")

            qT_sb = [cp.tile([128, 2, 512], dt.bfloat16, tag=f"qT{i}", name=f"qT{i}")
                     for i in range(QC)]
            kT_sb = [cp.tile([128, 2, 512], dt.bfloat16, tag=f"kT{i}", name=f"kT{i}")
                     for i in range(QC)]
            # v' blocks of 65 per head: v cols 0..63, ones col 64
            v_sb = [cp.tile([128, HC * 65], dt.bfloat16, tag=f"v{i}", name=f"v{i}")
                    for i in range(NKV)]

            # x staged as eighth-tiles [128, DT, 256] so projection chains can
            # start as soon as the first bytes land
            xq_t: list = [None] * 8
            xk_t: list = [None] * 8
            xv_t: list = [None] * 8

            def dma_x(xT, arr, e, nm, pool=None):
                t = (pool or xp).tile([128, DT, 256], dt.bfloat16, tag="xt",
                                      name=f"x_{nm}{e}")
                nc.sync.dma_start(
                    t[:],
                    xT.rearrange("(t p) s -> p t s", p=128)[:, :, e * 256:(e + 1) * 256])
                arr[e] = t

            def dma_w(w_sb, w, pt):
                nc.sync.dma_start(
                    w_sb[:, :, pt * 128:(pt + 1) * 128],
                    w.rearrange("(t p) m -> p t m", p=128)[:, :, pt * 128:(pt + 1) * 128])

            def kq_proj(xts, w_sb, dst, b_sb, qc, pt):
                ps = pm.tile([128, 512], dt.float32, tag="pm", name=f"pp{qc}_{pt}")
                for pc in range(2):
                    for Dti in range(DT):
                        nc.tensor.matmul(
                            ps[:, pc * 256:(pc + 1) * 256],
                            w_sb[:, Dti, pt * 128:(pt + 1) * 128],
                            xts[qc * 2 + pc][:, Dti, :],
                            start=(Dti == 0),
                            stop=(Dti == DT - 1),
                        )
                nc.vector.tensor_scalar_add(dst[qc][:, pt, :], ps[:], b_sb[:, pt:pt + 1])

            def v_chain(st, h):
                e, off = st // 2, (st % 2) * 128
                ps = pm.tile([128, HD], dt.float32, tag="pm", name=f"vp{st}_{h}")
                for Dti in range(DT):
                    nc.tensor.matmul(
                        ps[:],
                        xv_t[e][:, Dti, off:off + 128],
                        wv_sb[:, Dti, h * HD:(h + 1) * HD],
                        start=(Dti == 0),
                        stop=(Dti == DT - 1),
                    )
                nc.vector.tensor_tensor(
                    v_sb[st].rearrange("p (h c) -> p h c", c=65)[:, h, 0:64],
                    ps[:], bv_sb[:, h * HD:(h + 1) * HD], ALU.add)

            exs = {}

            def se(qc, h, kvb):
                pt, lo = h // 2, (h % 2) * 64
                scp = pa.tile([128, 1024], dt.float32, tag="pa", name=f"sc{qc}_{h}_{kvb}")
                for j in range(2):
                    kt = kvb * 2 + j
                    nc.tensor.matmul(
                        scp[:, j * 512:(j + 1) * 512],
                        kT_sb[kt // 4][lo:lo + 64, pt, (kt % 4) * 128:(kt % 4 + 1) * 128],
                        qT_sb[qc][lo:lo + 64, pt, :],
                        start=True,
                        stop=True,
                    )
                ex = ep.tile([128, 1024], dt.bfloat16, tag="ex", name=f"ex{qc}_{h}_{kvb}")
                nc.scalar.activation(ex[:], scp[:], AF.Exp, scale=0.125)
                exs[(qc, h, kvb)] = ex

            attn_n = {}

            def _burst_alloc(qc, h):
                acc = pacc.tile([128, HC * 65], dt.float32, tag="acc",
                                name=f"acc{qc}_{h}")
                rc = rp.tile([128, HC], dt.float32, tag="rc", name=f"rc{qc}_{h}")
                if h == 0:
                    for j in range(4):
                        attn_n[(qc, j)] = np_.tile(
                            [128, 256], dt.bfloat16, tag="an", name=f"an{qc}_{j}")
                return acc, rc

            def _burst_mm(qc, h, acc, kt):
                # one kv tile of attn[q, d]+sums: 4 interleaved accumulation
                # groups (one per q-subtile); ex is the stationary operand
                e = exs[(qc, h, kt // 2)]
                for j in range(4):
                    o = (kt % 2) * 512 + j * 128
                    nc.tensor.matmul(
                        acc[:, j * 65:j * 65 + 65],
                        e[:, o:o + 128],
                        v_sb[kt][:, h * 65:(h + 1) * 65],
                        start=(kt == 0),
                        stop=(kt == NKV - 1),
                        skip_group_check=True,
                    )
                if kt % 2 == 1:
                    exs.pop((qc, h, kt // 2))

            def _burst_norm(qc, h, acc, rc, j):
                accv = acc.rearrange("p (j c) -> p j c", c=65)
                nc.vector.reciprocal(rc[:, j:j + 1], accv[:, j, 64:65])
                nc.vector.tensor_scalar_mul(
                    attn_n[(qc, j)][:, h * 64:(h + 1) * 64],
                    accv[:, j, 0:64], rc[:, j:j + 1])

            def burst_works(qc, h):
                # 4 closures of 4 kv tiles each; kt-outer so ex tiles release
                # progressively (the next stage's exps reuse their slots)
                st = {}

                def mk(i):
                    def f():
                        if i == 0:
                            st["acc"], st["rc"] = _burst_alloc(qc, h)
                        acc = st["acc"]
                        for kt in range(NKV):
                            e = exs.get((qc, h, kt // 2))
                            o = (kt % 2) * 512 + i * 128
                            nc.tensor.matmul(
                                acc[:, i * 65:i * 65 + 65],
                                e[:, o:o + 128],
                                v_sb[kt][:, h * 65:(h + 1) * 65],
                                start=(kt == 0),
                                stop=(kt == NKV - 1),
                            )
                        _burst_norm(qc, h, acc, st["rc"], i)
                        if i == 3:
                            for kvb in range(8):
                                exs.pop((qc, h, kvb))
                    return f
                return [mk(i) for i in range(4)]

            def burst_whole(qc, h):
                for w in burst_works(qc, h):
                    w()

            at_t = {}

            def tr_j(qc, j):
                # transpose head pairs of q-subtile j: [128 q, 128] -> psum,
                # drained to at_t as the out-proj stationary
                att = attn_n[(qc, j)]
                att_t = ap_.tile([128, 256], dt.bfloat16, tag="at", name=f"at{qc}_{j}")
                for hp in range(2):
                    tr = pm.tile([128, 128], dt.bfloat16, tag="pm", name=f"tr{qc}_{j}_{hp}")
                    nc.tensor.transpose(tr[:], att[:, hp * 128:(hp + 1) * 128], id_sb[:])
                    nc.vector.tensor_copy(att_t[:, hp * 128:(hp + 1) * 128], tr[:])
                at_t[(qc, j)] = att_t

            def op_j(qc, j):
                # output projection row-slice for s-tile qc*4+j
                att_t = at_t.pop((qc, j))
                st = qc * 4 + j
                o_sb = op_.tile([128, D], dt.float32, tag="osb", name=f"o{st}")
                for dc2 in range(2):
                    po = pm.tile([128, 512], dt.float32, tag="pm", name=f"po{st}_{dc2}")
                    for hp in range(2):
                        nc.tensor.matmul(
                            po[:],
                            att_t[:, hp * 128:(hp + 1) * 128],
                            wo_sb[:, hp, dc2 * 512:(dc2 + 1) * 512],
                            start=(hp == 0),
                            stop=(hp == 1),
                        )
                    nc.vector.tensor_copy(o_sb[:, dc2 * 512:(dc2 + 1) * 512], po[:])
                    nc.sync.dma_start(
                        out_t[st][:, dc2 * 512:(dc2 + 1) * 512],
                        o_sb[:, dc2 * 512:(dc2 + 1) * 512])

            def tails(qc):
                # pipelined transpose -> out-proj works for a q-chunk; caller
                # weaves them between se units so the DVE drains are hidden
                return [
                    lambda qc=qc: tr_j(qc, 0),
                    lambda qc=qc: (op_j(qc, 0), tr_j(qc, 1)),
                    lambda qc=qc: (op_j(qc, 1), tr_j(qc, 2)),
                    lambda qc=qc: (op_j(qc, 2), tr_j(qc, 3)),
                    lambda qc=qc: op_j(qc, 3),
                ]

            def emit_block(ses, bws, extras):
                # One pipeline block. bws (a burst's 4 kt-group works) are
                # pinned just ahead of the se pair whose ex-slots they free:
                # [B0 se se B1 se se B2 se se B3 se se]; extras spread
                # order-preservingly into the remaining gaps.
                n = len(ses)
                post = {i: [] for i in range(-1, n)}
                if bws and n:
                    post[-1].append(bws[0])
                    for i, b in enumerate(bws[1:]):
                        post[2 * i + 1].append(b)
                    slots = [0, 2, 4, 6, 7]
                elif bws:
                    post[-1] = list(bws)
                    slots = [-1]
                else:
                    slots = list(range(n)) if n else [-1]
                for i, e in enumerate(extras):
                    post[slots[i * len(slots) // max(1, len(extras))]].append(e)
                for w in post[-1]:
                    w()
                for i, s in enumerate(ses):
                    se(*s)
                    for w in post[i]:
                        w()

            # ---- DMA emission (SP queue, consumption order; k/q first) ----
            dma_w(wk_sb, wk, 0)
            dma_x(xkT, xk_t, 0, "k")
            dma_x(xkT, xk_t, 1, "k")
            nc.sync.dma_start(bk_sb[:], bk[:])
            dma_w(wq_sb, wq, 0)
            dma_x(xqT, xq_t, 0, "q")
            dma_x(xqT, xq_t, 1, "q")
            nc.sync.dma_start(bq_sb[:], bq[:])
            for e in range(2, 8):
                dma_x(xkT, xk_t, e, "k")
            dma_x(xqT, xq_t, 2, "q")
            dma_x(xqT, xq_t, 3, "q")
            dma_w(wv_sb, wv, 0)
            dma_w(wv_sb, wv, 1)
            for e in range(4):
                dma_x(xvT, xv_t, e, "v", pool=xvp)
            nc.sync.dma_start(bv_sb[:], bv[:])
            for e in range(4, 8):
                dma_x(xvT, xv_t, e, "v", pool=xvp)
            dma_w(wk_sb, wk, 1)
            dma_w(wq_sb, wq, 1)
            nc.sync.dma_start(wo_sb[:], wo[:])
            nc.sync.dma_start(id_sb[:], ident[:])
            for e in range(4, 8):
                dma_x(xqT, xq_t, e, "q")

            for st in range(NKV):
                nc.vector.memset(
                    v_sb[st].rearrange("p (h c) -> p h c", c=65)[:, :, 64:65], 1.0)

            # ---- window 0: stage(0) + projections (pt0 first: heads 0/1
            # only read the pt0 half of kT/qT) ----
            kq_proj(xk_t, wk_sb, kT_sb, bk_sb, 0, 0)
            kq_proj(xq_t, wq_sb, qT_sb, bq_sb, 0, 0)
            se(0, 0, 0); se(0, 0, 1)
            kq_proj(xk_t, wk_sb, kT_sb, bk_sb, 1, 0)
            se(0, 0, 2); se(0, 0, 3)
            kq_proj(xk_t, wk_sb, kT_sb, bk_sb, 2, 0)
            se(0, 0, 4); se(0, 0, 5)
            kq_proj(xk_t, wk_sb, kT_sb, bk_sb, 3, 0)
            se(0, 0, 6); se(0, 0, 7)
            kq_proj(xq_t, wq_sb, qT_sb, bq_sb, 1, 0)

            def vch(st, h):
                return lambda: v_chain(st, h)

            def qp(q, pt):
                return lambda: kq_proj(xq_t, wq_sb, qT_sb, bq_sb, q, pt)

            def kp(c, pt):
                return lambda: kq_proj(xk_t, wk_sb, kT_sb, bk_sb, c, pt)

            # h1 block: vchains head 0 (first half)
            emit_block([(0, 1, kvb) for kvb in range(8)], [],
                       [vch(st, 0) for st in range(8)])
            kp(0, 1)(); kp(1, 1)(); qp(0, 1)()
            # h2 block: rest of head-0 vchains, head-1 vchains, pt1 kT chunks
            w_h2 = [vch(8, 0), vch(9, 0), kp(2, 1), vch(10, 0), kp(3, 1)]
            w_h2 += [vch(st, 0) for st in range(11, NKV)]
            w_h2 += [vch(st, 1) for st in range(8)]
            emit_block([(0, 2, kvb) for kvb in range(8)], [], w_h2)
            # h3 block: head-1/head-2 vchains + qT1 pt1
            w_h3 = [vch(st, 1) for st in range(8, NKV)]
            w_h3 += [qp(1, 1)]
            w_h3 += [vch(st, 2) for st in range(8)]
            emit_block([(0, 3, kvb) for kvb in range(8)], [], w_h3)

            # ---- steady blocks (qc, h): burst(qc, h) + se(qc+1, h, ·) ----
            for qc in range(3):
                for h in range(HC):
                    extras = []
                    if qc == 0:
                        if h == 0:
                            extras += [vch(st, 2) for st in range(8, NKV)]
                            extras += [vch(st, 3) for st in range(8)]
                        elif h == 1:
                            extras += [vch(st, 3) for st in range(8, NKV)]
                    if h == 0 and qc >= 1:
                        extras.append(qp(qc + 1, 1))
                    if qc == 2 and h >= 1:
                        # the last q-chunk's attn@V rides the exp stream
                        # instead of trailing it
                        extras += burst_works(3, h - 1)
                    if h == 3:
                        extras += tails(qc)
                        if qc < 2:
                            extras.append(qp(qc + 2, 0))
                    burst_whole(qc, h)
                    emit_block([(qc + 1, h, kvb) for kvb in range(8)],
                               [], extras)

            # final block: attn@V for (3,3) j-major so each q-subtile's
            # normalize/transpose fires as soon as its groups close
            acc, rc = _burst_alloc(3, 3)
            for j in range(4):
                for kt in range(NKV):
                    e = exs[(3, 3, kt // 2)]
                    o = (kt % 2) * 512 + j * 128
                    nc.tensor.matmul(
                        acc[:, j * 65:j * 65 + 65],
                        e[:, o:o + 128],
                        v_sb[kt][:, 3 * 65:4 * 65],
                        start=(kt == 0),
                        stop=(kt == NKV - 1),
                    )
                _burst_norm(3, 3, acc, rc, j)
                if j >= 1:
                    tr_j(3, j - 1)
            tr_j(3, 3)
            for j in range(4):
                op_j(3, j)

    nc.finalize()
    return nc


def _get_program():
    global _PROGRAM
    if _PROGRAM is None:
        _PROGRAM = _build_program()
    return _PROGRAM


def _prep_core_inputs(x_q, x_k, x_v, wq, bq, wk, bk, wv, bv, wo):
    """Build the 8 per-core input dicts (host-side shard + cast)."""
    ident_np = np.eye(128, dtype=np.float32).astype(BF16)
    xT = {}
    for b in range(2):
        xT[b] = (
            np.ascontiguousarray(x_q[b].T).astype(BF16),
            np.ascontiguousarray(x_k[b].T).astype(BF16),
            np.ascontiguousarray(x_v[b].T).astype(BF16),
        )
    in_maps = []
    for c in range(NCORES):
        b, g = c // 4, c % 4
        sl = slice(g * DC, (g + 1) * DC)
        # wo rows for this head group, stacked per head pair: row hh*64+d of
        # pair hp = wo row for head 2*hp+hh, dim d
        wo_c = np.ascontiguousarray(
            wo[sl, :].reshape(2, 2, HD, D).transpose(1, 2, 0, 3).reshape(128, 2, D)
        ).astype(BF16)
        in_maps.append({
            "xqT": xT[b][0],
            "xkT": xT[b][1],
            "xvT": xT[b][2],
            "wq": wq[:, sl].astype(BF16),
            "wk": wk[:, sl].astype(BF16),
            "wv": wv[:, sl].astype(BF16),
            "wo": wo_c,
            "bq": np.ascontiguousarray(bq[sl].reshape(2, 128).T).astype(np.float32),
            "bk": np.ascontiguousarray(bk[sl].reshape(2, 128).T).astype(np.float32),
            "bv": np.broadcast_to(bv[sl], (128, DC)).astype(np.float32).copy(),
            "ident": ident_np,
        })
    return in_maps


def kernel(x_q, x_k, x_v, wq, bq, wk, bk, wv, bv, wo, bo):
    from concourse.bass_utils import run_bass_kernel_spmd

    x_q = np.asarray(x_q, np.float32)
    x_k = np.asarray(x_k, np.float32)
    x_v = np.asarray(x_v, np.float32)
    wq = np.asarray(wq, np.float32)
    wk = np.asarray(wk, np.float32)
    wv = np.asarray(wv, np.float32)
    wo = np.asarray(wo, np.float32)
    bq = np.asarray(bq, np.float32)
    bk = np.asarray(bk, np.float32)
    bv = np.asarray(bv, np.float32)
    bo = np.asarray(bo, np.float32)

    nc = _get_program()
    in_maps = _prep_core_inputs(x_q, x_k, x_v, wq, bq, wk, bk, wv, bv, wo)
    res = run_bass_kernel_spmd(nc, in_maps, list(range(NCORES)))

    out = np.zeros((2, S, D), np.float32)
    for c in range(NCORES):
        out[c // 4] += res.results[c]["out"]
    out += bo
    return out


# revision 71
# speedup vs baseline: 1.1587x; 1.1583x over previous
"""Trainium2 Bass kernel for nn_MultiHeadAttention (B=2, S=2048, D=1024, H=16).

Sharding: 8 cores = 2 batches x 4 head-groups. Core c handles batch c//4 and
heads [4*(c%4), 4*(c%4)+4). Each core computes its 4 heads' attention plus the
row-slice of the output projection; the host sums the 4 partial outputs per
batch and adds the output bias.

Dataflow (cost model: matmul = N_out cycles regardless of M/K, so every
matmul keeps M=128 / K=128 where the math allows):
  - qT/kT in [head_dim, seq] layout, 2 heads per 128-partition tile.
  - scoresT[kv, q] = kT.T @ qT per (head, kv-pair, q-chunk); exp on ScalarE
    (scale=1/8 folded in) -> ex tiles [128 kv, 1024] bf16. The exp stream is
    the ACT-side bottleneck and paces the steady-state blocks.
  - attn[q, d+1] = ex.T @ [v | 1]: ex is the stationary operand (M=128 q,
    K=128 kv, N=65), accumulated over 16 kv tiles into PSUM [128, 4*65]
    per (head, q-chunk); col 64 of each head block = softmax denominator.
  - DVE reciprocal + per-partition tensor_scalar_mul normalizes into
    attn_n [128 q, 4*64] bf16 (q on partitions = denominators are
    per-partition scalars, no broadcast matmuls needed).
  - PE transpose (identity matmul) flips head-pairs [128 q, 128] ->
    [128 (2h*d), 128 q]; out = at.T @ wo accumulates K=128 (2 heads) per
    pass, halving the output projection.
  - Emission is a software pipeline of blocks (qc, h): two hoisted score
    units, then burst(qc, h) [attn@V for the previous stage, kt-outer with
    its four accumulation groups in four separate PSUM banks so ex tiles
    release progressively], then se(qc+1, h, ·) [scores+exp] woven with
    projections/tails so PE and ACT both stay dense. Stage 0 is emitted
    kvb-major (per-head ex rings keep the pools consistent) so the exp
    stream starts from the first kT/qT chunk; the last stage's bursts ride
    inside the last score blocks so only the final output projections
    trail the exp stream.

All matmuls run in bf16 (inputs cast on host) with fp32 PSUM accumulation.
"""

import sys

for _p in ("/opt/trn_rl_repo",):
    if _p not in sys.path:
        sys.path.insert(0, _p)

import numpy as np
import ml_dtypes

BF16 = ml_dtypes.bfloat16

S = 2048          # sequence length
D = 1024          # embed dim
HC = 4            # heads per core
HD = 64           # head dim
DC = HC * HD      # per-core projection width (256)
DT = D // 128     # D-tiles (8)
QC = S // 512     # q-chunks of 512 (4)
NKV = S // 128    # kv tiles of 128 (16)
NCORES = 8

_PROGRAM = None


def _build_program():
    import concourse.mybir as mybir
    import concourse.tile as tile
    from concourse import bacc

    dt = mybir.dt
    AF = mybir.ActivationFunctionType
    ALU = mybir.AluOpType

    nc = bacc.Bacc()

    xqT = nc.declare_dram_parameter("xqT", [D, S], dt.bfloat16, isOutput=False)
    xkT = nc.declare_dram_parameter("xkT", [D, S], dt.bfloat16, isOutput=False)
    xvT = nc.declare_dram_parameter("xvT", [D, S], dt.bfloat16, isOutput=False)
    wq = nc.declare_dram_parameter("wq", [D, DC], dt.bfloat16, isOutput=False)
    wk = nc.declare_dram_parameter("wk", [D, DC], dt.bfloat16, isOutput=False)
    wv = nc.declare_dram_parameter("wv", [D, DC], dt.bfloat16, isOutput=False)
    wo = nc.declare_dram_parameter("wo", [128, 2, D], dt.bfloat16, isOutput=False)
    bq = nc.declare_dram_parameter("bq", [128, 2], dt.float32, isOutput=False)
    bk = nc.declare_dram_parameter("bk", [128, 2], dt.float32, isOutput=False)
    bv = nc.declare_dram_parameter("bv", [128, DC], dt.float32, isOutput=False)
    ident = nc.declare_dram_parameter("ident", [128, 128], dt.bfloat16, isOutput=False)
    out = nc.declare_dram_parameter("out", [S, D], dt.bfloat16, isOutput=True)

    out_t = out.rearrange("(t p) d -> t p d", p=128)

    with tile.TileContext(nc) as tc:
        with (
            tc.tile_pool(name="const", bufs=1) as cp,
            tc.tile_pool(name="xt", bufs=2) as xp,
            tc.tile_pool(name="xv", bufs=8) as xvp,
            tc.tile_pool(name="expp", bufs=10) as ep,
            tc.tile_pool(name="anp", bufs=8) as np_,
            tc.tile_pool(name="atp", bufs=3) as ap_,
            tc.tile_pool(name="rcp", bufs=2) as rp,
            tc.tile_pool(name="outp", bufs=3) as op_,
            tc.tile_pool(name="pa", bufs=2, space="PSUM") as pa,
            tc.tile_pool(name="pacc", bufs=1, space="PSUM") as pacc,
            tc.tile_pool(name="pmix", bufs=3, space="PSUM") as pm,
        ):
            # ---- persistent tiles ----
            wq_sb = cp.tile([128, DT, DC], dt.bfloat16, tag="wq_sb")
            wk_sb = cp.tile([128, DT, DC], dt.bfloat16, tag="wk_sb")
            wv_sb = cp.tile([128, DT, DC], dt.bfloat16, tag="wv_sb")
            wo_sb = cp.tile([128, 2, D], dt.bfloat16, tag="wo_sb")
            bq_sb = cp.tile([128, 2], dt.float32, tag="bq_sb")
            bk_sb = cp.tile([128, 2], dt.float32, tag="bk_sb")
            bv_sb = cp.tile([128, DC], dt.float32, tag="bv_sb")
            id_sb = cp.tile([128, 128], dt.bfloat16, tag="id_sb")

            qT_sb = [cp.tile([128, 2, 512], dt.bfloat16, tag=f"qT{i}", name=f"qT{i}")
                     for i in range(QC)]
            kT_sb = [cp.tile([128, 2, 512], dt.bfloat16, tag=f"kT{i}", name=f"kT{i}")
                     for i in range(QC)]
            # v' blocks of 65 per head: v cols 0..63, ones col 64
            v_sb = [cp.tile([128, HC * 65], dt.bfloat16, tag=f"v{i}", name=f"v{i}")
                    for i in range(NKV)]

            # x staged as eighth-tiles [128, DT, 256] so projection chains can
            # start as soon as the first bytes land
            xq_t: list = [None] * 8
            xk_t: list = [None] * 8
            xv_t: list = [None] * 8

            def dma_x(xT, arr, e, nm, pool=None, split=False):
                # arr[e] = list of (tile, dti_base, dti_count) pieces
                xr = xT.rearrange("(t p) s -> p t s", p=128)
                if split:
                    # first tiles split by D-half so projection chains start
                    # as soon as the first bytes land
                    pieces = []
                    for s in range(2):
                        t = xp.tile([128, 4, 256], dt.bfloat16, tag="xs",
                                    bufs=16, name=f"x_{nm}{e}_{s}")
                        nc.sync.dma_start(
                            t[:], xr[:, s * 4:(s + 1) * 4, e * 256:(e + 1) * 256])
                        pieces.append((t, s * 4, 4))
                    arr[e] = pieces
                else:
                    t = (pool or xp).tile([128, DT, 256], dt.bfloat16, tag="xt",
                                          name=f"x_{nm}{e}")
                    nc.sync.dma_start(t[:], xr[:, :, e * 256:(e + 1) * 256])
                    arr[e] = [(t, 0, DT)]

            def dma_w(w_sb, w, pt):
                nc.sync.dma_start(
                    w_sb[:, :, pt * 128:(pt + 1) * 128],
                    w.rearrange("(t p) m -> p t m", p=128)[:, :, pt * 128:(pt + 1) * 128])

            def kq_proj(xts, w_sb, dst, b_sb, qc, pt):
                ps = pm.tile([128, 512], dt.float32, tag="pm", name=f"pp{qc}_{pt}")
                for pc in range(2):
                    n = 0
                    for t, dlo, dn in xts[qc * 2 + pc]:
                        for di in range(dn):
                            nc.tensor.matmul(
                                ps[:, pc * 256:(pc + 1) * 256],
                                w_sb[:, dlo + di, pt * 128:(pt + 1) * 128],
                                t[:, di, :],
                                start=(n == 0),
                                stop=(n == DT - 1),
                            )
                            n += 1
                nc.vector.tensor_scalar_add(dst[qc][:, pt, :], ps[:], b_sb[:, pt:pt + 1])

            def v_chain(st, h):
                e, off = st // 2, (st % 2) * 128
                ps = pm.tile([128, HD], dt.float32, tag="pm", name=f"vp{st}_{h}")
                n = 0
                for t, dlo, dn in xv_t[e]:
                    for di in range(dn):
                        nc.tensor.matmul(
                            ps[:],
                            t[:, di, off:off + 128],
                            wv_sb[:, dlo + di, h * HD:(h + 1) * HD],
                            start=(n == 0),
                            stop=(n == DT - 1),
                        )
                        n += 1
                nc.vector.tensor_tensor(
                    v_sb[st].rearrange("p (h c) -> p h c", c=65)[:, h, 0:64],
                    ps[:], bv_sb[:, h * HD:(h + 1) * HD], ALU.add)

            exs = {}

            def se(qc, h, kvb):
                pt, lo = h // 2, (h % 2) * 64
                scp = pa.tile([128, 1024], dt.float32, tag="pa", name=f"sc{qc}_{h}_{kvb}")
                for j in range(2):
                    kt = kvb * 2 + j
                    nc.tensor.matmul(
                        scp[:, j * 512:(j + 1) * 512],
                        kT_sb[kt // 4][lo:lo + 64, pt, (kt % 4) * 128:(kt % 4 + 1) * 128],
                        qT_sb[qc][lo:lo + 64, pt, :],
                        start=True,
                        stop=True,
                    )
                ex = ep.tile([128, 1024], dt.bfloat16, tag=f"ex{h}", bufs=10,
                             name=f"ex{qc}_{h}_{kvb}")
                nc.scalar.activation(ex[:], scp[:], AF.Exp, scale=0.125)
                exs[(qc, h, kvb)] = ex

            attn_n = {}

            def _burst_alloc(qc, h):
                # the last stage's bursts ride inside other blocks while a
                # regular burst is in flight, so they draw from the pm pool
                pool, tag = (pm, "pm") if qc == 3 else (pacc, "acc")
                acc = pool.tile([128, HC * 65], dt.float32, tag=tag,
                                name=f"acc{qc}_{h}")
                rc = rp.tile([128, HC], dt.float32, tag="rc", name=f"rc{qc}_{h}")
                if h == 0:
                    for j in range(4):
                        attn_n[(qc, j)] = np_.tile(
                            [128, 256], dt.bfloat16, tag="an", name=f"an{qc}_{j}")
                return acc, rc

            def _burst_mm(qc, h, acc, kt):
                # one kv tile of attn[q, d]+sums: 4 interleaved accumulation
                # groups (one per q-subtile); ex is the stationary operand
                e = exs[(qc, h, kt // 2)]
                for j in range(4):
                    o = (kt % 2) * 512 + j * 128
                    nc.tensor.matmul(
                        acc[:, j * 65:j * 65 + 65],
                        e[:, o:o + 128],
                        v_sb[kt][:, h * 65:(h + 1) * 65],
                        start=(kt == 0),
                        stop=(kt == NKV - 1),
                        skip_group_check=True,
                    )
                if kt % 2 == 1:
                    exs.pop((qc, h, kt // 2))

            def _burst_norm(qc, h, acc, rc, j):
                accv = acc.rearrange("p (j c) -> p j c", c=65)
                nc.vector.reciprocal(rc[:, j:j + 1], accv[:, j, 64:65])
                nc.vector.tensor_scalar_mul(
                    attn_n[(qc, j)][:, h * 64:(h + 1) * 64],
                    accv[:, j, 0:64], rc[:, j:j + 1])

            def burst_works(qc, h):
                # 4 closures of 4 kv tiles each; kt-outer so ex tiles release
                # progressively (the next stage's exps reuse their slots)
                st = {}

                def mk(i):
                    def f():
                        if i == 0:
                            st["acc"], st["rc"] = _burst_alloc(qc, h)
                        acc = st["acc"]
                        for kt in range(NKV):
                            e = exs.get((qc, h, kt // 2))
                            o = (kt % 2) * 512 + i * 128
                            nc.tensor.matmul(
                                acc[:, i * 65:i * 65 + 65],
                                e[:, o:o + 128],
                                v_sb[kt][:, h * 65:(h + 1) * 65],
                                start=(kt == 0),
                                stop=(kt == NKV - 1),
                            )
                        _burst_norm(qc, h, acc, st["rc"], i)
                        if i == 3:
                            for kvb in range(8):
                                exs.pop((qc, h, kvb))
                    return f
                return [mk(i) for i in range(4)]

            def burst_whole(qc, h):
                # kt-outer with the four q-subtile accumulation groups in four
                # separate PSUM banks (one group per bank): ex tiles release
                # progressively so the next stage's exps flow without waiting
                # for the whole burst
                accs = [pacc.tile([128, 65], dt.float32, tag="acc",
                                  name=f"acc{qc}_{h}_0")]
                accs += [pm.tile([128, 65], dt.float32, tag="pm",
                                 name=f"acc{qc}_{h}_{j}") for j in range(1, 4)]
                rc = rp.tile([128, HC], dt.float32, tag="rc", name=f"rc{qc}_{h}")
                if h == 0:
                    for j in range(4):
                        attn_n[(qc, j)] = np_.tile(
                            [128, 256], dt.bfloat16, tag="an", name=f"an{qc}_{j}")
                for kt in range(NKV):
                    e = exs[(qc, h, kt // 2)]
                    for j in range(4):
                        o = (kt % 2) * 512 + j * 128
                        nc.tensor.matmul(
                            accs[j][:, 0:65],
                            e[:, o:o + 128],
                            v_sb[kt][:, h * 65:(h + 1) * 65],
                            start=(kt == 0),
                            stop=(kt == NKV - 1),
                        )
                    if kt % 2 == 1:
                        exs.pop((qc, h, kt // 2))
                for j in range(4):
                    nc.vector.reciprocal(rc[:, j:j + 1], accs[j][:, 64:65])
                    nc.vector.tensor_scalar_mul(
                        attn_n[(qc, j)][:, h * 64:(h + 1) * 64],
                        accs[j][:, 0:64], rc[:, j:j + 1])

            at_t = {}

            def tr_j(qc, j, act_drain=False):
                # transpose head pairs of q-subtile j: [128 q, 128] -> psum,
                # drained to at_t as the out-proj stationary
                att = attn_n[(qc, j)]
                att_t = ap_.tile([128, 256], dt.bfloat16, tag="at", name=f"at{qc}_{j}")
                for hp in range(2):
                    tr = pm.tile([128, 128], dt.bfloat16, tag="pm", name=f"tr{qc}_{j}_{hp}")
                    nc.tensor.transpose(tr[:], att[:, hp * 128:(hp + 1) * 128], id_sb[:])
                    if act_drain and hp == 0:
                        nc.scalar.copy(att_t[:, hp * 128:(hp + 1) * 128], tr[:])
                    else:
                        nc.vector.tensor_copy(att_t[:, hp * 128:(hp + 1) * 128], tr[:])
                at_t[(qc, j)] = att_t

            def op_j(qc, j, act_drain=False, po_pool=None):
                # output projection row-slice for s-tile qc*4+j
                att_t = at_t.pop((qc, j))
                st = qc * 4 + j
                o_sb = op_.tile([128, D], dt.bfloat16, tag="osb", name=f"o{st}")
                pool, tg = (po_pool, "pa") if po_pool is not None else (pm, "pm")
                for dc2 in range(2):
                    po = pool.tile([128, 512], dt.float32, tag=tg, name=f"po{st}_{dc2}")
                    for hp in range(2):
                        nc.tensor.matmul(
                            po[:],
                            att_t[:, hp * 128:(hp + 1) * 128],
                            wo_sb[:, hp, dc2 * 512:(dc2 + 1) * 512],
                            start=(hp == 0),
                            stop=(hp == 1),
                        )
                    if act_drain and dc2 == 0:
                        nc.scalar.copy(o_sb[:, dc2 * 512:(dc2 + 1) * 512], po[:])
                    else:
                        nc.vector.tensor_copy(o_sb[:, dc2 * 512:(dc2 + 1) * 512], po[:])
                    if not act_drain:
                        nc.sync.dma_start(
                            out_t[st][:, dc2 * 512:(dc2 + 1) * 512],
                            o_sb[:, dc2 * 512:(dc2 + 1) * 512])
                if act_drain:
                    # endgame: one whole-tile DMA (the final out-DMAs are
                    # SEQ-paced, not transfer-paced)
                    nc.sync.dma_start(out_t[st][:], o_sb[:])

            def tails(qc):
                # pipelined transpose -> out-proj works for a q-chunk; caller
                # weaves them between se units so the DVE drains are hidden
                return [
                    lambda qc=qc: tr_j(qc, 0),
                    lambda qc=qc: (op_j(qc, 0), tr_j(qc, 1)),
                    lambda qc=qc: (op_j(qc, 1), tr_j(qc, 2)),
                    lambda qc=qc: (op_j(qc, 2), tr_j(qc, 3)),
                    lambda qc=qc: op_j(qc, 3),
                ]

            def emit_block(ses, bws, extras):
                # One pipeline block. bws (a burst's 4 kt-group works) are
                # pinned just ahead of the se pair whose ex-slots they free:
                # [B0 se se B1 se se B2 se se B3 se se]; extras spread
                # order-preservingly into the remaining gaps.
                n = len(ses)
                post = {i: [] for i in range(-1, n)}
                if bws and n:
                    post[-1].append(bws[0])
                    for i, b in enumerate(bws[1:]):
                        post[2 * i + 1].append(b)
                    slots = [1, 3, 5, 7]
                elif bws:
                    post[-1] = list(bws)
                    slots = [-1]
                else:
                    # extras go after odd se indices only: the pa ring frees
                    # slots in pairs, and delaying the refill se starves ACT
                    slots = [i for i in range(n) if i % 2 == 1] or ([0] if n else [-1])
                for i, e in enumerate(extras):
                    post[slots[i * len(slots) // max(1, len(extras))]].append(e)
                for w in post[-1]:
                    w()
                for i, s in enumerate(ses):
                    se(*s)
                    for w in post[i]:
                        w()

            # ---- DMA emission (SP queue, consumption order; k/q first) ----
            dma_w(wk_sb, wk, 0)
            dma_x(xkT, xk_t, 0, "k", split=True)
            dma_x(xkT, xk_t, 1, "k", split=True)
            dma_w(wq_sb, wq, 0)
            dma_x(xqT, xq_t, 0, "q", split=True)
            nc.sync.dma_start(bq_sb[:], bq[:])
            dma_x(xqT, xq_t, 1, "q", split=True)
            nc.sync.dma_start(bk_sb[:], bk[:])
            dma_w(wk_sb, wk, 1)
            dma_w(wq_sb, wq, 1)
            dma_x(xkT, xk_t, 2, "k", split=True)
            dma_x(xkT, xk_t, 3, "k", split=True)
            for e in range(4, 8):
                dma_x(xkT, xk_t, e, "k", split=True)
            dma_x(xqT, xq_t, 2, "q")
            dma_x(xqT, xq_t, 3, "q")
            dma_w(wv_sb, wv, 0)
            dma_w(wv_sb, wv, 1)
            nc.sync.dma_start(bv_sb[:], bv[:])
            for e in range(8):
                dma_x(xvT, xv_t, e, "v", pool=xvp)
            nc.sync.dma_start(wo_sb[:], wo[:])
            nc.sync.dma_start(id_sb[:], ident[:])
            for e in range(4, 8):
                dma_x(xqT, xq_t, e, "q")

            for st in range(NKV):
                nc.gpsimd.memset(
                    v_sb[st].rearrange("p (h c) -> p h c", c=65)[:, :, 64:65], 1.0)

            def vch(st, h):
                return lambda: v_chain(st, h)

            def qp(q, pt):
                return lambda: kq_proj(xq_t, wq_sb, qT_sb, bq_sb, q, pt)

            def kp(c, pt):
                return lambda: kq_proj(xk_t, wk_sb, kT_sb, bk_sb, c, pt)

            # ---- window 0: stage(0) kvb-major so ACT runs dense from the
            # first kT/qT chunk (both pt halves of chunk 0 feed 8 exps before
            # chunk 1 is even needed); per-head ex rings keep this
            # ring-consistent with the h-major steady blocks ----
            kp(0, 0)(); qp(0, 0)()
            se(0, 0, 0); se(0, 1, 0)
            kp(0, 1)()
            se(0, 0, 1); se(0, 1, 1)
            qp(0, 1)()
            se(0, 2, 0); se(0, 3, 0)
            kp(1, 0)()
            se(0, 2, 1); se(0, 3, 1)
            kp(1, 1)()
            for kvb in (2, 3):
                for h in range(HC):
                    se(0, h, kvb)
                if kvb == 2:
                    kp(2, 0)()
                else:
                    kp(2, 1)()
            vst = 0
            for kvb in (4, 5):
                for h in range(HC):
                    se(0, h, kvb)
                    if h == 1:
                        (kp(3, 0) if kvb == 4 else kp(3, 1))()
                    elif vst < 4:
                        v_chain(vst, 0)
                        vst += 1
            for kvb in (6, 7):
                for h in range(HC):
                    se(0, h, kvb)
                    if h == 1:
                        (qp(1, 0) if kvb == 6 else qp(1, 1))()
                    else:
                        for _ in range(2):
                            if vst < NKV:
                                v_chain(vst, 0)
                                vst += 1
            while vst < NKV:
                v_chain(vst, 0)
                vst += 1

            # ---- steady blocks (qc, h): burst(qc, h) + se(qc+1, h, ·) ----
            for qc in range(3):
                for h in range(HC):
                    extras = []
                    if qc == 0 and h < 3:
                        # vchains one block ahead of the burst that needs them
                        extras += [vch(st, h + 1) for st in range(NKV)]
                    if h == 0 and qc >= 1:
                        # spill the previous q-chunk's last tail works here to
                        # keep block (qc-1, 3) under the ACT pace
                        extras += [lambda q=qc - 1: (op_j(q, 2), tr_j(q, 3)),
                                   lambda q=qc - 1: op_j(q, 3)]
                        extras.append(qp(qc + 1, 1))
                    if qc == 2 and h in (1, 2):
                        # the last q-chunk's attn@V rides the exp stream
                        # instead of trailing it
                        extras += burst_works(3, h - 1)
                    if h == 3:
                        if qc < 2:
                            extras += tails(qc)[0:3]
                            extras.append(qp(qc + 2, 0))
                        else:
                            # keep block (2,3) balanced: only the first two
                            # q-subtiles' tails; the rest rides the endgame
                            extras += [lambda: tr_j(2, 0),
                                       lambda: (op_j(2, 0), tr_j(2, 1)),
                                       lambda: op_j(2, 1)]
                    # two se units hoisted ahead of the burst (legal thanks to
                    # the ep pool's 4 spare slots): ACT chews them while the
                    # burst owns the PE
                    se(qc + 1, h, 0)
                    se(qc + 1, h, 1)
                    burst_whole(qc, h)
                    emit_block([(qc + 1, h, kvb) for kvb in range(2, 8)],
                               [], extras)

            # ---- endgame: b(3,2) + leftover tails fill PE while the last
            # exps stream; b(3,3) runs kt-outer with its four q-subtile
            # accumulation groups in four separate PSUM banks so only the
            # final kv pair trails the last exp ----
            b32 = burst_works(3, 2)
            b32[0](); tr_j(2, 2)
            b32[1](); op_j(2, 2)
            b32[2](); tr_j(2, 3)
            b32[3](); op_j(2, 3)

            acc3 = [pacc.tile([128, 65], dt.float32, tag="acc", name="acc33_0")]
            acc3 += [pm.tile([128, 65], dt.float32, tag="pm", name=f"acc33_{j}")
                     for j in range(1, 4)]
            rc3 = rp.tile([128, HC], dt.float32, tag="rc", name="rc33")
            for kt in range(NKV):
                e = exs[(3, 3, kt // 2)]
                for j in range(4):
                    o = (kt % 2) * 512 + j * 128
                    nc.tensor.matmul(
                        acc3[j][:, 0:65],
                        e[:, o:o + 128],
                        v_sb[kt][:, 3 * 65:4 * 65],
                        start=(kt == 0),
                        stop=(kt == NKV - 1),
                    )
            for j in range(4):
                nc.vector.reciprocal(rc3[:, j:j + 1], acc3[j][:, 64:65])
                nc.vector.tensor_scalar_mul(
                    attn_n[(3, j)][:, 3 * 64:4 * 64],
                    acc3[j][:, 0:64], rc3[:, j:j + 1])
            # the scp pool's banks are dead after the last exp: alternate the
            # final po tiles into them so the out-proj cadence never waits a
            # drain roundtrip
            tr_j(3, 0, act_drain=True)
            tr_j(3, 1, act_drain=True)
            op_j(3, 0, act_drain=True)
            tr_j(3, 2, act_drain=True)
            op_j(3, 1, act_drain=True, po_pool=pa)
            tr_j(3, 3, act_drain=True)
            op_j(3, 2, act_drain=True)
            op_j(3, 3, act_drain=True, po_pool=pa)

    nc.finalize()
    return nc


def _get_program():
    global _PROGRAM
    if _PROGRAM is None:
        _PROGRAM = _build_program()
    return _PROGRAM


def _prep_core_inputs(x_q, x_k, x_v, wq, bq, wk, bk, wv, bv, wo):
    """Build the 8 per-core input dicts (host-side shard + cast)."""
    ident_np = np.eye(128, dtype=np.float32).astype(BF16)
    xT = {}
    for b in range(2):
        xT[b] = (
            np.ascontiguousarray(x_q[b].T).astype(BF16),
            np.ascontiguousarray(x_k[b].T).astype(BF16),
            np.ascontiguousarray(x_v[b].T).astype(BF16),
        )
    in_maps = []
    for c in range(NCORES):
        b, g = c // 4, c % 4
        sl = slice(g * DC, (g + 1) * DC)
        # wo rows for this head group, stacked per head pair: row hh*64+d of
        # pair hp = wo row for head 2*hp+hh, dim d
        wo_c = np.ascontiguousarray(
            wo[sl, :].reshape(2, 2, HD, D).transpose(1, 2, 0, 3).reshape(128, 2, D)
        ).astype(BF16)
        in_maps.append({
            "xqT": xT[b][0],
            "xkT": xT[b][1],
            "xvT": xT[b][2],
            "wq": wq[:, sl].astype(BF16),
            "wk": wk[:, sl].astype(BF16),
            "wv": wv[:, sl].astype(BF16),
            "wo": wo_c,
            "bq": np.ascontiguousarray(bq[sl].reshape(2, 128).T).astype(np.float32),
            "bk": np.ascontiguousarray(bk[sl].reshape(2, 128).T).astype(np.float32),
            "bv": np.broadcast_to(bv[sl], (128, DC)).astype(np.float32).copy(),
            "ident": ident_np,
        })
    return in_maps


def kernel(x_q, x_k, x_v, wq, bq, wk, bk, wv, bv, wo, bo):
    from concourse.bass_utils import run_bass_kernel_spmd

    x_q = np.asarray(x_q, np.float32)
    x_k = np.asarray(x_k, np.float32)
    x_v = np.asarray(x_v, np.float32)
    wq = np.asarray(wq, np.float32)
    wk = np.asarray(wk, np.float32)
    wv = np.asarray(wv, np.float32)
    wo = np.asarray(wo, np.float32)
    bq = np.asarray(bq, np.float32)
    bk = np.asarray(bk, np.float32)
    bv = np.asarray(bv, np.float32)
    bo = np.asarray(bo, np.float32)

    nc = _get_program()
    in_maps = _prep_core_inputs(x_q, x_k, x_v, wq, bq, wk, bk, wv, bv, wo)
    res = run_bass_kernel_spmd(nc, in_maps, list(range(NCORES)))

    out = np.zeros((2, S, D), np.float32)
    for c in range(NCORES):
        out[c // 4] += np.asarray(res.results[c]["out"], dtype=np.float32)
    out += bo
    return out


# revision 72
# speedup vs baseline: 1.1590x; 1.0003x over previous
"""Trainium2 Bass kernel for nn_MultiHeadAttention (B=2, S=2048, D=1024, H=16).

Sharding: 8 cores = 2 batches x 4 head-groups. Core c handles batch c//4 and
heads [4*(c%4), 4*(c%4)+4). Each core computes its 4 heads' attention plus the
row-slice of the output projection; the host sums the 4 partial outputs per
batch and adds the output bias.

Dataflow (cost model: matmul = N_out cycles regardless of M/K, so every
matmul keeps M=128 / K=128 where the math allows):
  - qT/kT in [head_dim, seq] layout, 2 heads per 128-partition tile.
  - scoresT[kv, q] = kT.T @ qT per (head, kv-pair, q-chunk); exp on ScalarE
    (scale=1/8 folded in) -> ex tiles [128 kv, 1024] bf16. The exp stream is
    the ACT-side bottleneck and paces the steady-state blocks.
  - attn[q, d+1] = ex.T @ [v | 1]: ex is the stationary operand (M=128 q,
    K=128 kv, N=65), accumulated over 16 kv tiles into PSUM [128, 4*65]
    per (head, q-chunk); col 64 of each head block = softmax denominator.
  - DVE reciprocal + per-partition tensor_scalar_mul normalizes into
    attn_n [128 q, 4*64] bf16 (q on partitions = denominators are
    per-partition scalars, no broadcast matmuls needed).
  - PE transpose (identity matmul) flips head-pairs [128 q, 128] ->
    [128 (2h*d), 128 q]; out = at.T @ wo accumulates K=128 (2 heads) per
    pass, halving the output projection.
  - Emission is a software pipeline of blocks (qc, h): two hoisted score
    units, then burst(qc, h) [attn@V for the previous stage, kt-outer with
    its four accumulation groups in four separate PSUM banks so ex tiles
    release progressively], then se(qc+1, h, ·) [scores+exp] woven with
    projections/tails so PE and ACT both stay dense. Stage 0 is emitted
    kvb-major (per-head ex rings keep the pools consistent) so the exp
    stream starts from the first kT/qT chunk; the last stage's bursts ride
    inside the last score blocks so only the final output projections
    trail the exp stream.

All matmuls run in bf16 (inputs cast on host) with fp32 PSUM accumulation.
"""

import sys

for _p in ("/opt/trn_rl_repo",):
    if _p not in sys.path:
        sys.path.insert(0, _p)

import numpy as np
import ml_dtypes

BF16 = ml_dtypes.bfloat16

S = 2048          # sequence length
D = 1024          # embed dim
HC = 4            # heads per core
HD = 64           # head dim
DC = HC * HD      # per-core projection width (256)
DT = D // 128     # D-tiles (8)
QC = S // 512     # q-chunks of 512 (4)
NKV = S // 128    # kv tiles of 128 (16)
NCORES = 8

_PROGRAM = None


def _build_program():
    import concourse.mybir as mybir
    import concourse.tile as tile
    from concourse import bacc

    dt = mybir.dt
    AF = mybir.ActivationFunctionType
    ALU = mybir.AluOpType

    nc = bacc.Bacc()

    xqT = nc.declare_dram_parameter("xqT", [D, S], dt.bfloat16, isOutput=False)
    xkT = nc.declare_dram_parameter("xkT", [D, S], dt.bfloat16, isOutput=False)
    xvT = nc.declare_dram_parameter("xvT", [D, S], dt.bfloat16, isOutput=False)
    wq = nc.declare_dram_parameter("wq", [D, DC], dt.bfloat16, isOutput=False)
    wk = nc.declare_dram_parameter("wk", [D, DC], dt.bfloat16, isOutput=False)
    wv = nc.declare_dram_parameter("wv", [D, DC], dt.bfloat16, isOutput=False)
    wo = nc.declare_dram_parameter("wo", [128, 2, D], dt.bfloat16, isOutput=False)
    bq = nc.declare_dram_parameter("bq", [128, 2], dt.float32, isOutput=False)
    bk = nc.declare_dram_parameter("bk", [128, 2], dt.float32, isOutput=False)
    bv = nc.declare_dram_parameter("bv", [128, DC], dt.float32, isOutput=False)
    ident = nc.declare_dram_parameter("ident", [128, 128], dt.bfloat16, isOutput=False)
    out = nc.declare_dram_parameter("out", [S, D], dt.bfloat16, isOutput=True)

    out_t = out.rearrange("(t p) d -> t p d", p=128)

    with tile.TileContext(nc) as tc:
        with (
            tc.tile_pool(name="const", bufs=1) as cp,
            tc.tile_pool(name="xt", bufs=2) as xp,
            tc.tile_pool(name="xv", bufs=8) as xvp,
            tc.tile_pool(name="expp", bufs=10) as ep,
            tc.tile_pool(name="anp", bufs=8) as np_,
            tc.tile_pool(name="atp", bufs=3) as ap_,
            tc.tile_pool(name="rcp", bufs=2) as rp,
            tc.tile_pool(name="outp", bufs=3) as op_,
            tc.tile_pool(name="pa", bufs=2, space="PSUM") as pa,
            tc.tile_pool(name="pacc", bufs=1, space="PSUM") as pacc,
            tc.tile_pool(name="pmix", bufs=3, space="PSUM") as pm,
        ):
            # ---- persistent tiles ----
            wq_sb = cp.tile([128, DT, DC], dt.bfloat16, tag="wq_sb")
            wk_sb = cp.tile([128, DT, DC], dt.bfloat16, tag="wk_sb")
            wv_sb = cp.tile([128, DT, DC], dt.bfloat16, tag="wv_sb")
            wo_sb = cp.tile([128, 2, D], dt.bfloat16, tag="wo_sb")
            bq_sb = cp.tile([128, 2], dt.float32, tag="bq_sb")
            bk_sb = cp.tile([128, 2], dt.float32, tag="bk_sb")
            bv_sb = cp.tile([128, DC], dt.float32, tag="bv_sb")
            id_sb = cp.tile([128, 128], dt.bfloat16, tag="id_sb")

            qT_sb = [cp.tile([128, 2, 512], dt.bfloat16, tag=f"qT{i}", name=f"qT{i}")
                     for i in range(QC)]
            kT_sb = [cp.tile([128, 2, 512], dt.bfloat16, tag=f"kT{i}", name=f"kT{i}")
                     for i in range(QC)]
            # v' blocks of 65 per head: v cols 0..63, ones col 64
            v_sb = [cp.tile([128, HC * 65], dt.bfloat16, tag=f"v{i}", name=f"v{i}")
                    for i in range(NKV)]

            # x staged as eighth-tiles [128, DT, 256] so projection chains can
            # start as soon as the first bytes land
            xq_t: list = [None] * 8
            xk_t: list = [None] * 8
            xv_t: list = [None] * 8

            def dma_x(xT, arr, e, nm, pool=None, split=False):
                # arr[e] = list of (tile, dti_base, dti_count) pieces
                xr = xT.rearrange("(t p) s -> p t s", p=128)
                if split:
                    # first tiles split by D-half so projection chains start
                    # as soon as the first bytes land
                    pieces = []
                    for s in range(2):
                        t = xp.tile([128, 4, 256], dt.bfloat16, tag="xs",
                                    bufs=16, name=f"x_{nm}{e}_{s}")
                        nc.sync.dma_start(
                            t[:], xr[:, s * 4:(s + 1) * 4, e * 256:(e + 1) * 256])
                        pieces.append((t, s * 4, 4))
                    arr[e] = pieces
                else:
                    t = (pool or xp).tile([128, DT, 256], dt.bfloat16, tag="xt",
                                          name=f"x_{nm}{e}")
                    nc.sync.dma_start(t[:], xr[:, :, e * 256:(e + 1) * 256])
                    arr[e] = [(t, 0, DT)]

            def dma_w(w_sb, w, pt):
                nc.sync.dma_start(
                    w_sb[:, :, pt * 128:(pt + 1) * 128],
                    w.rearrange("(t p) m -> p t m", p=128)[:, :, pt * 128:(pt + 1) * 128])

            def kq_proj(xts, w_sb, dst, b_sb, qc, pt):
                ps = pm.tile([128, 512], dt.float32, tag="pm", name=f"pp{qc}_{pt}")
                for pc in range(2):
                    n = 0
                    for t, dlo, dn in xts[qc * 2 + pc]:
                        for di in range(dn):
                            nc.tensor.matmul(
                                ps[:, pc * 256:(pc + 1) * 256],
                                w_sb[:, dlo + di, pt * 128:(pt + 1) * 128],
                                t[:, di, :],
                                start=(n == 0),
                                stop=(n == DT - 1),
                            )
                            n += 1
                nc.vector.tensor_scalar_add(dst[qc][:, pt, :], ps[:], b_sb[:, pt:pt + 1])

            def v_chain(st, h):
                e, off = st // 2, (st % 2) * 128
                ps = pm.tile([128, HD], dt.float32, tag="pm", name=f"vp{st}_{h}")
                n = 0
                for t, dlo, dn in xv_t[e]:
                    for di in range(dn):
                        nc.tensor.matmul(
                            ps[:],
                            t[:, di, off:off + 128],
                            wv_sb[:, dlo + di, h * HD:(h + 1) * HD],
                            start=(n == 0),
                            stop=(n == DT - 1),
                        )
                        n += 1
                nc.vector.tensor_tensor(
                    v_sb[st].rearrange("p (h c) -> p h c", c=65)[:, h, 0:64],
                    ps[:], bv_sb[:, h * HD:(h + 1) * HD], ALU.add)

            exs = {}

            def se(qc, h, kvb):
                pt, lo = h // 2, (h % 2) * 64
                scp = pa.tile([128, 1024], dt.float32, tag="pa", name=f"sc{qc}_{h}_{kvb}")
                for j in range(2):
                    kt = kvb * 2 + j
                    nc.tensor.matmul(
                        scp[:, j * 512:(j + 1) * 512],
                        kT_sb[kt // 4][lo:lo + 64, pt, (kt % 4) * 128:(kt % 4 + 1) * 128],
                        qT_sb[qc][lo:lo + 64, pt, :],
                        start=True,
                        stop=True,
                    )
                ex = ep.tile([128, 1024], dt.bfloat16, tag=f"ex{h}", bufs=10,
                             name=f"ex{qc}_{h}_{kvb}")
                nc.scalar.activation(ex[:], scp[:], AF.Exp, scale=0.125)
                exs[(qc, h, kvb)] = ex

            attn_n = {}

            def _burst_alloc(qc, h):
                # the last stage's bursts ride inside other blocks while a
                # regular burst is in flight, so they draw from the pm pool
                pool, tag = (pm, "pm") if qc == 3 else (pacc, "acc")
                acc = pool.tile([128, HC * 65], dt.float32, tag=tag,
                                name=f"acc{qc}_{h}")
                rc = rp.tile([128, HC], dt.float32, tag="rc", name=f"rc{qc}_{h}")
                if h == 0:
                    for j in range(4):
                        attn_n[(qc, j)] = np_.tile(
                            [128, 256], dt.bfloat16, tag="an", name=f"an{qc}_{j}")
                return acc, rc

            def _burst_mm(qc, h, acc, kt):
                # one kv tile of attn[q, d]+sums: 4 interleaved accumulation
                # groups (one per q-subtile); ex is the stationary operand
                e = exs[(qc, h, kt // 2)]
                for j in range(4):
                    o = (kt % 2) * 512 + j * 128
                    nc.tensor.matmul(
                        acc[:, j * 65:j * 65 + 65],
                        e[:, o:o + 128],
                        v_sb[kt][:, h * 65:(h + 1) * 65],
                        start=(kt == 0),
                        stop=(kt == NKV - 1),
                        skip_group_check=True,
                    )
                if kt % 2 == 1:
                    exs.pop((qc, h, kt // 2))

            def _burst_norm(qc, h, acc, rc, j):
                accv = acc.rearrange("p (j c) -> p j c", c=65)
                nc.vector.reciprocal(rc[:, j:j + 1], accv[:, j, 64:65])
                nc.vector.tensor_scalar_mul(
                    attn_n[(qc, j)][:, h * 64:(h + 1) * 64],
                    accv[:, j, 0:64], rc[:, j:j + 1])

            def burst_works(qc, h):
                # 4 closures of 4 kv tiles each; kt-outer so ex tiles release
                # progressively (the next stage's exps reuse their slots)
                st = {}

                def mk(i):
                    def f():
                        if i == 0:
                            st["acc"], st["rc"] = _burst_alloc(qc, h)
                        acc = st["acc"]
                        for kt in range(NKV):
                            e = exs.get((qc, h, kt // 2))
                            o = (kt % 2) * 512 + i * 128
                            nc.tensor.matmul(
                                acc[:, i * 65:i * 65 + 65],
                                e[:, o:o + 128],
                                v_sb[kt][:, h * 65:(h + 1) * 65],
                                start=(kt == 0),
                                stop=(kt == NKV - 1),
                            )
                        _burst_norm(qc, h, acc, st["rc"], i)
                        if i == 3:
                            for kvb in range(8):
                                exs.pop((qc, h, kvb))
                    return f
                return [mk(i) for i in range(4)]

            def burst_whole(qc, h):
                # kt-outer with the four q-subtile accumulation groups in four
                # separate PSUM banks (one group per bank): ex tiles release
                # progressively so the next stage's exps flow without waiting
                # for the whole burst
                accs = [pacc.tile([128, 65], dt.float32, tag="acc",
                                  name=f"acc{qc}_{h}_0")]
                accs += [pm.tile([128, 65], dt.float32, tag="pm",
                                 name=f"acc{qc}_{h}_{j}") for j in range(1, 4)]
                rc = rp.tile([128, HC], dt.float32, tag="rc", name=f"rc{qc}_{h}")
                if h == 0:
                    for j in range(4):
                        attn_n[(qc, j)] = np_.tile(
                            [128, 256], dt.bfloat16, tag="an", name=f"an{qc}_{j}")
                for kt in range(NKV):
                    e = exs[(qc, h, kt // 2)]
                    for j in range(4):
                        o = (kt % 2) * 512 + j * 128
                        nc.tensor.matmul(
                            accs[j][:, 0:65],
                            e[:, o:o + 128],
                            v_sb[kt][:, h * 65:(h + 1) * 65],
                            start=(kt == 0),
                            stop=(kt == NKV - 1),
                        )
                    if kt % 2 == 1:
                        exs.pop((qc, h, kt // 2))
                for j in range(4):
                    nc.vector.reciprocal(rc[:, j:j + 1], accs[j][:, 64:65])
                    nc.vector.tensor_scalar_mul(
                        attn_n[(qc, j)][:, h * 64:(h + 1) * 64],
                        accs[j][:, 0:64], rc[:, j:j + 1])

            at_t = {}

            def tr_j(qc, j, act_drain=False):
                # transpose head pairs of q-subtile j: [128 q, 128] -> psum,
                # drained to at_t as the out-proj stationary
                att = attn_n[(qc, j)]
                att_t = ap_.tile([128, 256], dt.bfloat16, tag="at", name=f"at{qc}_{j}")
                for hp in range(2):
                    tr = pm.tile([128, 128], dt.bfloat16, tag="pm", name=f"tr{qc}_{j}_{hp}")
                    nc.tensor.transpose(tr[:], att[:, hp * 128:(hp + 1) * 128], id_sb[:])
                    if act_drain and hp == 0:
                        nc.scalar.copy(att_t[:, hp * 128:(hp + 1) * 128], tr[:])
                    else:
                        nc.vector.tensor_copy(att_t[:, hp * 128:(hp + 1) * 128], tr[:])
                at_t[(qc, j)] = att_t

            def op_j(qc, j, act_drain=False, po_pool=None):
                # output projection row-slice for s-tile qc*4+j
                att_t = at_t.pop((qc, j))
                st = qc * 4 + j
                o_sb = op_.tile([128, D], dt.bfloat16, tag="osb", name=f"o{st}")
                pool, tg = (po_pool, "pa") if po_pool is not None else (pm, "pm")
                for dc2 in range(2):
                    po = pool.tile([128, 512], dt.float32, tag=tg, name=f"po{st}_{dc2}")
                    for hp in range(2):
                        nc.tensor.matmul(
                            po[:],
                            att_t[:, hp * 128:(hp + 1) * 128],
                            wo_sb[:, hp, dc2 * 512:(dc2 + 1) * 512],
                            start=(hp == 0),
                            stop=(hp == 1),
                        )
                    if act_drain and dc2 == 0:
                        nc.scalar.copy(o_sb[:, dc2 * 512:(dc2 + 1) * 512], po[:])
                    else:
                        nc.vector.tensor_copy(o_sb[:, dc2 * 512:(dc2 + 1) * 512], po[:])
                    if not act_drain:
                        nc.sync.dma_start(
                            out_t[st][:, dc2 * 512:(dc2 + 1) * 512],
                            o_sb[:, dc2 * 512:(dc2 + 1) * 512])
                if act_drain:
                    # endgame: one whole-tile DMA (the final out-DMAs are
                    # SEQ-paced, not transfer-paced)
                    nc.sync.dma_start(out_t[st][:], o_sb[:])

            def tails(qc):
                # pipelined transpose -> out-proj works for a q-chunk; caller
                # weaves them between se units so the DVE drains are hidden
                return [
                    lambda qc=qc: tr_j(qc, 0),
                    lambda qc=qc: (op_j(qc, 0), tr_j(qc, 1)),
                    lambda qc=qc: (op_j(qc, 1), tr_j(qc, 2)),
                    lambda qc=qc: (op_j(qc, 2), tr_j(qc, 3)),
                    lambda qc=qc: op_j(qc, 3),
                ]

            def emit_block(ses, bws, extras):
                # One pipeline block. bws (a burst's 4 kt-group works) are
                # pinned just ahead of the se pair whose ex-slots they free:
                # [B0 se se B1 se se B2 se se B3 se se]; extras spread
                # order-preservingly into the remaining gaps.
                n = len(ses)
                post = {i: [] for i in range(-1, n)}
                if bws and n:
                    post[-1].append(bws[0])
                    for i, b in enumerate(bws[1:]):
                        post[2 * i + 1].append(b)
                    slots = [1, 3, 5, 7]
                elif bws:
                    post[-1] = list(bws)
                    slots = [-1]
                else:
                    # extras go after odd se indices only: the pa ring frees
                    # slots in pairs, and delaying the refill se starves ACT
                    slots = [i for i in range(n) if i % 2 == 1] or ([0] if n else [-1])
                for i, e in enumerate(extras):
                    post[slots[i * len(slots) // max(1, len(extras))]].append(e)
                for w in post[-1]:
                    w()
                for i, s in enumerate(ses):
                    se(*s)
                    for w in post[i]:
                        w()

            # ---- DMA emission (SP queue, consumption order; k/q first) ----
            dma_w(wk_sb, wk, 0)
            dma_x(xkT, xk_t, 0, "k", split=True)
            dma_x(xkT, xk_t, 1, "k", split=True)
            dma_w(wq_sb, wq, 0)
            dma_x(xqT, xq_t, 0, "q", split=True)
            nc.sync.dma_start(bq_sb[:], bq[:])
            dma_x(xqT, xq_t, 1, "q", split=True)
            nc.sync.dma_start(bk_sb[:], bk[:])
            dma_w(wk_sb, wk, 1)
            dma_w(wq_sb, wq, 1)
            dma_x(xkT, xk_t, 2, "k", split=True)
            dma_x(xkT, xk_t, 3, "k", split=True)
            for e in range(4, 8):
                dma_x(xkT, xk_t, e, "k", split=True)
            dma_x(xqT, xq_t, 2, "q")
            dma_x(xqT, xq_t, 3, "q")
            dma_w(wv_sb, wv, 0)
            dma_w(wv_sb, wv, 1)
            nc.sync.dma_start(bv_sb[:], bv[:])
            for e in range(8):
                dma_x(xvT, xv_t, e, "v", pool=xvp)
            nc.sync.dma_start(wo_sb[:], wo[:])
            nc.sync.dma_start(id_sb[:], ident[:])
            for e in range(4, 8):
                dma_x(xqT, xq_t, e, "q")

            for st in range(NKV):
                nc.gpsimd.memset(
                    v_sb[st].rearrange("p (h c) -> p h c", c=65)[:, :, 64:65], 1.0)

            def vch(st, h):
                return lambda: v_chain(st, h)

            def qp(q, pt):
                return lambda: kq_proj(xq_t, wq_sb, qT_sb, bq_sb, q, pt)

            def kp(c, pt):
                return lambda: kq_proj(xk_t, wk_sb, kT_sb, bk_sb, c, pt)

            # ---- window 0: stage(0) kvb-major so ACT runs dense from the
            # first kT/qT chunk (both pt halves of chunk 0 feed 8 exps before
            # chunk 1 is even needed); per-head ex rings keep this
            # ring-consistent with the h-major steady blocks ----
            kp(0, 0)(); qp(0, 0)()
            se(0, 0, 0); se(0, 1, 0)
            kp(0, 1)()
            se(0, 0, 1); se(0, 1, 1)
            qp(0, 1)()
            se(0, 2, 0); se(0, 3, 0)
            kp(1, 0)()
            se(0, 2, 1); se(0, 3, 1)
            kp(1, 1)()
            for kvb in (2, 3):
                for h in range(HC):
                    se(0, h, kvb)
                if kvb == 2:
                    kp(2, 0)()
                else:
                    kp(2, 1)()
            vst = 0
            for kvb in (4, 5):
                for h in range(HC):
                    se(0, h, kvb)
                    if h == 1:
                        (kp(3, 0) if kvb == 4 else kp(3, 1))()
                    elif vst < 4:
                        v_chain(vst, 0)
                        vst += 1
            for kvb in (6, 7):
                for h in range(HC):
                    se(0, h, kvb)
                    if h == 1:
                        (qp(1, 0) if kvb == 6 else qp(1, 1))()
                    else:
                        for _ in range(2):
                            if vst < NKV:
                                v_chain(vst, 0)
                                vst += 1
            while vst < NKV:
                v_chain(vst, 0)
                vst += 1

            # ---- steady blocks (qc, h): burst(qc, h) + se(qc+1, h, ·) ----
            for qc in range(3):
                for h in range(HC):
                    extras = []
                    if qc == 0 and h < 3:
                        # vchains one block ahead of the burst that needs them
                        extras += [vch(st, h + 1) for st in range(NKV)]
                    if h == 0 and qc >= 1:
                        # spill the previous q-chunk's last tail works here to
                        # keep block (qc-1, 3) under the ACT pace
                        extras += [lambda q=qc - 1: (op_j(q, 2), tr_j(q, 3)),
                                   lambda q=qc - 1: op_j(q, 3)]
                        extras.append(qp(qc + 1, 1))
                    if qc == 2 and h in (1, 2):
                        # the last q-chunk's attn@V rides the exp stream
                        # instead of trailing it
                        extras += burst_works(3, h - 1)
                    if h == 3:
                        if qc < 2:
                            extras += tails(qc)[0:3]
                            extras.append(qp(qc + 2, 0))
                        else:
                            # keep block (2,3) balanced: only the first two
                            # q-subtiles' tails; the rest rides the endgame
                            extras += [lambda: tr_j(2, 0),
                                       lambda: (op_j(2, 0), tr_j(2, 1)),
                                       lambda: op_j(2, 1)]
                    # two se units hoisted ahead of the burst (legal thanks to
                    # the ep pool's 4 spare slots): ACT chews them while the
                    # burst owns the PE
                    se(qc + 1, h, 0)
                    se(qc + 1, h, 1)
                    burst_whole(qc, h)
                    emit_block([(qc + 1, h, kvb) for kvb in range(2, 8)],
                               [], extras)

            # ---- endgame: b(3,2) + leftover tails fill PE while the last
            # exps stream; b(3,3) runs kt-outer with its four q-subtile
            # accumulation groups in four separate PSUM banks so only the
            # final kv pair trails the last exp ----
            b32 = burst_works(3, 2)
            b32[0](); tr_j(2, 2)
            b32[1](); op_j(2, 2)
            b32[2](); tr_j(2, 3)
            b32[3](); op_j(2, 3)

            acc3 = [pacc.tile([128, 65], dt.float32, tag="acc", name="acc33_0")]
            acc3 += [pm.tile([128, 65], dt.float32, tag="pm", name=f"acc33_{j}")
                     for j in range(1, 4)]
            rc3 = rp.tile([128, HC], dt.float32, tag="rc", name="rc33")
            for kt in range(NKV):
                e = exs[(3, 3, kt // 2)]
                for j in range(4):
                    o = (kt % 2) * 512 + j * 128
                    nc.tensor.matmul(
                        acc3[j][:, 0:65],
                        e[:, o:o + 128],
                        v_sb[kt][:, 3 * 65:4 * 65],
                        start=(kt == 0),
                        stop=(kt == NKV - 1),
                    )
            for j in range(4):
                nc.vector.reciprocal(rc3[:, j:j + 1], acc3[j][:, 64:65])
                if j < 2:
                    nc.vector.tensor_scalar_mul(
                        attn_n[(3, j)][:, 3 * 64:4 * 64],
                        acc3[j][:, 0:64], rc3[:, j:j + 1])
                else:
                    # ACT is idle post-exp: normalize j2/j3 there so the
                    # transpose/out-proj chain starts sooner
                    nc.scalar.activation(
                        attn_n[(3, j)][:, 3 * 64:4 * 64],
                        acc3[j][:, 0:64], AF.Copy, scale=rc3[:, j:j + 1])
            # the scp pool's banks are dead after the last exp: alternate the
            # final po tiles into them so the out-proj cadence never waits a
            # drain roundtrip
            tr_j(3, 0, act_drain=True)
            tr_j(3, 1, act_drain=True)
            op_j(3, 0, act_drain=True)
            tr_j(3, 2, act_drain=True)
            op_j(3, 1, act_drain=True, po_pool=pa)
            tr_j(3, 3, act_drain=True)
            op_j(3, 2, act_drain=True)
            op_j(3, 3, act_drain=True, po_pool=pa)

    nc.finalize()
    return nc


def _get_program():
    global _PROGRAM
    if _PROGRAM is None:
        _PROGRAM = _build_program()
    return _PROGRAM


def _prep_core_inputs(x_q, x_k, x_v, wq, bq, wk, bk, wv, bv, wo):
    """Build the 8 per-core input dicts (host-side shard + cast)."""
    ident_np = np.eye(128, dtype=np.float32).astype(BF16)
    xT = {}
    for b in range(2):
        xT[b] = (
            np.ascontiguousarray(x_q[b].T).astype(BF16),
            np.ascontiguousarray(x_k[b].T).astype(BF16),
            np.ascontiguousarray(x_v[b].T).astype(BF16),
        )
    in_maps = []
    for c in range(NCORES):
        b, g = c // 4, c % 4
        sl = slice(g * DC, (g + 1) * DC)
        # wo rows for this head group, stacked per head pair: row hh*64+d of
        # pair hp = wo row for head 2*hp+hh, dim d
        wo_c = np.ascontiguousarray(
            wo[sl, :].reshape(2, 2, HD, D).transpose(1, 2, 0, 3).reshape(128, 2, D)
        ).astype(BF16)
        in_maps.append({
            "xqT": xT[b][0],
            "xkT": xT[b][1],
            "xvT": xT[b][2],
            "wq": wq[:, sl].astype(BF16),
            "wk": wk[:, sl].astype(BF16),
            "wv": wv[:, sl].astype(BF16),
            "wo": wo_c,
            "bq": np.ascontiguousarray(bq[sl].reshape(2, 128).T).astype(np.float32),
            "bk": np.ascontiguousarray(bk[sl].reshape(2, 128).T).astype(np.float32),
            "bv": np.broadcast_to(bv[sl], (128, DC)).astype(np.float32).copy(),
            "ident": ident_np,
        })
    return in_maps


def kernel(x_q, x_k, x_v, wq, bq, wk, bk, wv, bv, wo, bo):
    from concourse.bass_utils import run_bass_kernel_spmd

    x_q = np.asarray(x_q, np.float32)
    x_k = np.asarray(x_k, np.float32)
    x_v = np.asarray(x_v, np.float32)
    wq = np.asarray(wq, np.float32)
    wk = np.asarray(wk, np.float32)
    wv = np.asarray(wv, np.float32)
    wo = np.asarray(wo, np.float32)
    bq = np.asarray(bq, np.float32)
    bk = np.asarray(bk, np.float32)
    bv = np.asarray(bv, np.float32)
    bo = np.asarray(bo, np.float32)

    nc = _get_program()
    in_maps = _prep_core_inputs(x_q, x_k, x_v, wq, bq, wk, bk, wv, bv, wo)
    res = run_bass_kernel_spmd(nc, in_maps, list(range(NCORES)))

    out = np.zeros((2, S, D), np.float32)
    for c in range(NCORES):
        out[c // 4] += np.asarray(res.results[c]["out"], dtype=np.float32)
    out += bo
    return out
